# revision 9
# baseline (speedup 1.0000x reference)
"""Trainium2 Bass kernel: single-token GQA decode layer + gated MLP,
Megatron-style tensor-parallel across 8 NeuronCores.

Self-contained: hardcodes all shapes. kernel(**inputs) takes the full
(unsharded) inputs and returns the full (1, HID, 1, 1) output.

Per-core shard (core c of 8):
  - attention: q heads [4c, 4c+4), kv head c, Wo columns [512c, 512c+512)
  - MLP: intermediate slice [1536c, 1536c+1536) of gate/up/down
  - one on-device AllReduce (f32, 16KB) after o_proj; the down_proj
    partials are summed on the host (each core returns hidden/8 + mlp_i).

All weights are pre-transposed + tiled on the host into [128, ...] slabs
with the contraction index on the partition dim, so every DMA descriptor
is a large contiguous run and every matmul is lhsT=[K=128, M=128] (or
the activation vector as rhs, N=1).

KV-cache scatter at current_pos is handled without any device-side
dynamic indexing: the host zeroes column pos of k^T and row pos of v,
bakes exp(causal_mask) and a one-hot(pos) column into an augmented
V slab [v*emask | emask | onehot*emask], and the device adds the new
token's contribution exp(q.k_new) (x) [v_new | 1 | 0] into the same
PSUM accumulator, then corrects the softmax denominator with the
one-hot column.
"""

import numpy as np
import ml_dtypes

import concourse.bass as bass
import concourse.bacc as bacc
import concourse.mybir as mybir
import concourse.tile as tile
from concourse.bass_utils import run_bass_kernel_spmd

# ---- problem dims (hardcoded) ----
H = 32
HKV = 8
G = H // HKV          # 4 q heads per kv head
D = 128
HID = 4096            # = 32 * 128
INT = 12288           # per-core intermediate = 1536 = 12 * 128
S = 8192              # = 64 * 128
EPS = 1e-6
SCALE = 1.0 / float(np.sqrt(D))
NCORES = 8

HK = HID // 128       # 32 k-tiles over hidden
QR = G * D            # 512 q rows per core
NM_QKV = (QR + 2 * D) // 128   # 6 m-tiles (4 q + 1 k + 1 v)
NM_O = HID // 128     # 32
NKO = QR // 128       # 4 k-tiles for o_proj
IPC = INT // NCORES   # 1536 per-core intermediate
NM_GU = 2 * IPC // 128  # 24 m-tiles (12 gate + 12 up)
NK_D = IPC // 128     # 12 k-tiles for down proj
NS = S // 128         # 64 s-tiles

# compute dtype for weights / big matmuls ("bf16" or "f32")
COMPUTE = "bf16"

_cache = {}


def _wdt():
    return mybir.dt.bfloat16 if COMPUTE == "bf16" else mybir.dt.float32


def _npwdt():
    return ml_dtypes.bfloat16 if COMPUTE == "bf16" else np.float32


# ---------------------------------------------------------------- builder
def _build():
    f32 = mybir.dt.float32
    wdt = _wdt()
    bf = COMPUTE == "bf16"

    nc = bacc.Bacc(
        "TRN2",
        target_bir_lowering=False,
        debug=False,
        enable_asserts=False,
        num_devices=NCORES,
    )

    def inp(name, shape, dtype=f32):
        return nc.dram_tensor(name, list(shape), dtype, kind="ExternalInput").ap()

    x_d = inp("x", (128, HK))
    lnw1_d = inp("lnw1", (128, HK))
    lnw2_d = inp("lnw2", (128, HK))
    qkn_d = inp("qknw", (5, 128))
    cos_d = inp("cos5", (5, 128))
    sin_d = inp("sin5", (5, 128))
    cmpos_d = inp("cmpos", (1, 1))
    ident_d = inp("ident", (128, 128))
    wqkv_d = inp("wqkv", (128, NM_QKV * HK * 128), wdt)
    wo_d = inp("wo", (128, NM_O * NKO * 128), wdt)
    wgu_d = inp("wgu", (128, NM_GU * HK * 128), wdt)
    wd_d = inp("wd", (128, NM_O * NK_D * 128), wdt)
    kt_d = inp("kt", (128, S), wdt)
    vaug_d = inp("vaug", (128, NS * 130), wdt)
    out_d = nc.dram_tensor("out", [128, HK], f32, kind="ExternalOutput").ap()

    AF = mybir.ActivationFunctionType
    OP = mybir.AluOpType

    with tile.TileContext(nc) as tc:
        with (
            tc.tile_pool(name="persist", bufs=1) as pp,
            tc.tile_pool(name="wq_s", bufs=3 if bf else 2) as wqkv_pool,
            tc.tile_pool(name="kt_s", bufs=4 if bf else 2) as kt_pool,
            tc.tile_pool(name="va_s", bufs=4 if bf else 2) as va_pool,
            tc.tile_pool(name="wo_s", bufs=3 if bf else 2) as wo_pool,
            tc.tile_pool(name="wgu_s", bufs=8 if bf else 3) as wgu_pool,
            tc.tile_pool(name="wd_s", bufs=4 if bf else 2) as wd_pool,
            tc.tile_pool(name="e_s", bufs=4) as e_pool,
            tc.tile_pool(name="ps_small", bufs=2, space="PSUM") as ps_small,
            tc.tile_pool(name="ps_mm", bufs=2, space="PSUM") as ps_mm,
            tc.tile_pool(name="ps_sc", bufs=2, space="PSUM") as ps_sc_pool,
            tc.tile_pool(name="ps_acc", bufs=1, space="PSUM") as ps_acc_pool,
            tc.tile_pool(name="dram", bufs=1, space="DRAM") as dram_pool,
        ):
            # ---------- small constant loads (issued first, sync ring) ----
            x_sb = pp.tile([128, HK], f32)
            lnw1 = pp.tile([128, HK], f32)
            lnw2 = pp.tile([128, HK], f32)
            qknw = pp.tile([5, 128], f32)
            cos5 = pp.tile([5, 128], f32)
            sin5 = pp.tile([5, 128], f32)
            cmpos = pp.tile([1, 1], f32)
            ident = pp.tile([128, 128], f32)
            nc.sync.dma_start(x_sb[:], x_d[:])
            nc.sync.dma_start(lnw1[:], lnw1_d[:])
            nc.sync.dma_start(lnw2[:], lnw2_d[:])
            nc.sync.dma_start(qknw[:], qkn_d[:])
            nc.sync.dma_start(cos5[:], cos_d[:])
            nc.sync.dma_start(sin5[:], sin_d[:])
            nc.sync.dma_start(cmpos[:], cmpos_d[:])
            nc.sync.dma_start(ident[:], ident_d[:])

            # attention-phase streams on the sync (SP) HWDGE ring
            wqkv_t = []
            for m in range(NM_QKV):
                wq = wqkv_pool.tile([128, HK * 128], wdt, tag="wqkv_c")
                nc.sync.dma_start(
                    wq[:], wqkv_d[:, m * HK * 128:(m + 1) * HK * 128]
                )
                wqkv_t.append(wq)
            kt_t = []
            for b in range(4):
                kt = kt_pool.tile([128, 2048], wdt, tag="kt_c")
                nc.sync.dma_start(kt[:], kt_d[:, b * 2048:(b + 1) * 2048])
                kt_t.append(kt)
            va_t = []
            for b in range(4):
                va = va_pool.tile([128, 16 * 130], wdt, tag="va_c")
                nc.sync.dma_start(va[:], vaug_d[:, b * 2080:(b + 1) * 2080])
                va_t.append(va)
            wo_t = []
            for ch in range(4):
                wo = wo_pool.tile([128, 4096], wdt, tag="wo_c")
                nc.sync.dma_start(wo[:], wo_d[:, ch * 4096:(ch + 1) * 4096])
                wo_t.append(wo)

            # MLP weight streams on the scalar (ACT) HWDGE ring
            wgu_t = []
            for m in range(NM_GU):
                wg = wgu_pool.tile([128, HK * 128], wdt, tag="wgu_c")
                nc.scalar.dma_start(
                    wg[:], wgu_d[:, m * HK * 128:(m + 1) * HK * 128]
                )
                wgu_t.append(wg)
            wd_t = []
            for ch in range(16):
                wd = wd_pool.tile([128, 2 * NK_D * 128], wdt, tag="wd_c")
                nc.scalar.dma_start(
                    wd[:], wd_d[:, ch * 3072:(ch + 1) * 3072]
                )
                wd_t.append(wd)

            ones_col = pp.tile([128, 1], f32)
            ones_row = pp.tile([1, 128], f32)
            eps_col = pp.tile([128, 1], f32)
            zero_col = pp.tile([128, 1], f32)
            nc.vector.memset(ones_col[:], 1.0)
            nc.vector.memset(ones_row[:], 1.0)
            nc.vector.memset(eps_col[:], EPS)
            nc.vector.memset(zero_col[:], 0.0)

            # ---------- rmsnorm(x) -> hb ------------------------------
            def rmsnorm(src, lnw, out_tile, nbuf):
                sq = pp.tile([128, HK], f32, tag="rms_sq", name=f"sq{nbuf}")
                rs = pp.tile([128, 1], f32, tag="rms_rs", name=f"rs{nbuf}")
                nc.vector.tensor_mul(sq[:], src[:], src[:])
                nc.vector.reduce_sum(rs[:], sq[:], axis=mybir.AxisListType.X)
                ps_tot = ps_small.tile([1, 1], f32, tag="small", name=f"pt{nbuf}")
                nc.tensor.matmul(ps_tot[:], rs[:], ones_col[:], start=True, stop=True)
                sd = pp.tile([1, 1], f32, tag="rms_sd", name=f"sd{nbuf}")
                nc.scalar.activation(
                    sd[:], ps_tot[:], AF.Sqrt, bias=eps_col[0:1, 0:1],
                    scale=1.0 / HID
                )
                inv = pp.tile([1, 1], f32, tag="rms_inv", name=f"inv{nbuf}")
                nc.vector.reciprocal(inv[:], sd[:])
                ps_b = ps_small.tile([128, 1], f32, tag="small", name=f"pb{nbuf}")
                nc.tensor.matmul(ps_b[:], ones_row[:], inv[:], start=True, stop=True)
                invb = pp.tile([128, 1], f32, tag="rms_invb", name=f"invb{nbuf}")
                nc.vector.tensor_copy(invb[:], ps_b[:])
                # out = (src * invb) * lnw, cast to compute dtype
                nc.vector.scalar_tensor_tensor(
                    out_tile[:], src[:], invb[:, 0:1], lnw[:], OP.mult, OP.mult
                )

            hb = pp.tile([128, HK], wdt)
            rmsnorm(x_sb, lnw1, hb, 0)

            # ---------- qkv projection --------------------------------
            ps_qkv = ps_mm.tile([128, NM_QKV], f32, tag="mm", name="ps_qkv")
            for m in range(NM_QKV):
                for k in range(HK):
                    nc.tensor.matmul(
                        ps_qkv[:, m:m + 1],
                        wqkv_t[m][:, k * 128:(k + 1) * 128],
                        hb[:, k:k + 1],
                        start=(k == 0),
                        stop=(k == HK - 1),
                    )

            # ---------- transpose to [head, d], norm + rope -----------
            qkvT = pp.tile([128, NM_QKV], f32)
            nc.vector.tensor_copy(qkvT[:], ps_qkv[:])
            ps_t1 = ps_small.tile([NM_QKV, 128], f32, tag="small", name="ps_t1")
            nc.tensor.transpose(ps_t1[:], qkvT[:], ident[:])
            qkv_hd = pp.tile([NM_QKV, 128], f32)
            nc.vector.tensor_copy(qkv_hd[:], ps_t1[:])

            # rmsnorm rows 0..4 (4 q heads + k) over free dim
            sq5 = pp.tile([5, 128], f32)
            rs5 = pp.tile([5, 1], f32)
            nc.vector.tensor_mul(sq5[:], qkv_hd[0:5, :], qkv_hd[0:5, :])
            nc.vector.reduce_sum(rs5[:], sq5[:], axis=mybir.AxisListType.X)
            sd5 = pp.tile([5, 1], f32)
            nc.scalar.activation(
                sd5[:], rs5[:], AF.Sqrt, bias=eps_col[0:5, 0:1], scale=1.0 / D
            )
            inv5 = pp.tile([5, 1], f32)
            nc.vector.reciprocal(inv5[:], sd5[:])
            qkn = pp.tile([5, 128], f32)
            nc.vector.scalar_tensor_tensor(
                qkn[:], qkv_hd[0:5, :], inv5[:, 0:1], qknw[:], OP.mult, OP.mult
            )
            # rope: rot = [-x[64:], x[:64]]
            rot = pp.tile([5, 128], f32)
            nc.vector.tensor_scalar_mul(rot[:, 0:64], qkn[:, 64:128], -1.0)
            nc.vector.tensor_copy(rot[:, 64:128], qkn[:, 0:64])
            t1 = pp.tile([5, 128], f32)
            nc.vector.tensor_mul(t1[:], qkn[:], cos5[:])
            qk_r = pp.tile([5, 128], f32)
            # qk_r = rot*sin + t1
            nc.vector.scalar_tensor_tensor(
                qk_r[:], rot[:], 1.0, sin5[:], OP.bypass, OP.mult
            )
            nc.vector.tensor_add(qk_r[:], qk_r[:], t1[:])

            # transpose back -> qkT [d=128, 5] (cols 0-3 q heads, col 4 k_new)
            ps_t2 = ps_small.tile([128, 5], f32, tag="small", name="ps_t2")
            nc.tensor.transpose(ps_t2[:], qk_r[:], ident[0:5, 0:5])
            qkT = pp.tile([128, 5], wdt)
            nc.vector.tensor_copy(qkT[:], ps_t2[:])

            # vnew_aug [1, 130] = [v_new | 1 | 0]; extract v row via a
            # transpose of qkvT column 5 (partition slices must be aligned,
            # so qkv_hd[5:6, :] cannot be read directly)
            ps_vt = ps_small.tile([1, 128], f32, tag="small", name="ps_vt")
            nc.tensor.transpose(ps_vt[:], qkvT[:, 5:6], ident[:])
            vnew = pp.tile([1, 130], wdt)
            nc.vector.memset(vnew[:], 0.0)
            nc.vector.tensor_copy(vnew[0:1, 0:128], ps_vt[:])
            nc.vector.memset(vnew[0:1, 128:129], 1.0)

            # e_new^T [1, 4] = exp(SCALE * q.k_new + cm[pos])
            ps_en = ps_small.tile([1, 4], f32, tag="small", name="ps_en")
            nc.tensor.matmul(
                ps_en[:], qkT[:, 4:5], qkT[:, 0:4], start=True, stop=True
            )
            enT = pp.tile([1, 4], wdt)
            nc.scalar.activation(
                enT[:], ps_en[:], AF.Exp, bias=cmpos[0:1, 0:1], scale=SCALE
            )

            # ---------- scores + exp ----------------------------------
            e_t = []
            for b in range(4):
                ps_sc = ps_sc_pool.tile([128, 64], f32, tag="sc", name=f"sc{b}")
                for t in range(16):
                    nc.tensor.matmul(
                        ps_sc[:, 4 * t:4 * t + 4],
                        kt_t[b][:, t * 128:(t + 1) * 128],
                        qkT[:, 0:4],
                        start=True,
                        stop=True,
                    )
                e_sb = e_pool.tile([128, 64], wdt, tag="e_c", name=f"e{b}")
                nc.scalar.activation(
                    e_sb[:], ps_sc[:], AF.Exp, bias=zero_col[:, 0:1], scale=SCALE
                )
                e_t.append(e_sb)

            # ---------- attn @ v_aug ----------------------------------
            ps_acc = ps_acc_pool.tile([4, 130], f32)
            first = True
            for b in range(4):
                for t in range(16):
                    nc.tensor.matmul(
                        ps_acc[:],
                        e_t[b][:, 4 * t:4 * t + 4],
                        va_t[b][:, t * 130:(t + 1) * 130],
                        start=first,
                        stop=False,
                    )
                    first = False
            nc.tensor.matmul(ps_acc[:], enT[:], vnew[:], start=False, stop=True)

            acc_sb = pp.tile([4, 130], f32)
            nc.vector.tensor_copy(acc_sb[:], ps_acc[:])
            den = pp.tile([4, 1], f32)
            nc.vector.tensor_sub(den[:], acc_sb[:, 128:129], acc_sb[:, 129:130])
            rec = pp.tile([4, 1], f32)
            nc.vector.reciprocal(rec[:], den[:])
            attn = pp.tile([4, 128], f32)
            nc.vector.tensor_scalar_mul(attn[:], acc_sb[:, 0:128], rec[:, 0:1])
            ps_t3 = ps_small.tile([128, 4], f32, tag="small", name="ps_t3")
            nc.tensor.transpose(ps_t3[:], attn[:], ident[0:4, 0:4])
            attnT = pp.tile([128, 4], wdt)
            nc.vector.tensor_copy(attnT[:], ps_t3[:])

            # ---------- o_proj partial --------------------------------
            ps_p = ps_mm.tile([128, NM_O], f32, tag="mm", name="ps_p")
            for ch in range(4):
                for mm in range(8):
                    m = ch * 8 + mm
                    for j in range(NKO):
                        nc.tensor.matmul(
                            ps_p[:, m:m + 1],
                            wo_t[ch][:, (mm * NKO + j) * 128:(mm * NKO + j + 1) * 128],
                            attnT[:, j:j + 1],
                            start=(j == 0),
                            stop=(j == NKO - 1),
                        )
            p_sb = pp.tile([128, NM_O], f32)
            nc.vector.tensor_copy(p_sb[:], ps_p[:])

            # ---------- AllReduce o_proj partials ---------------------
            ar_in = dram_pool.tile([128, NM_O], f32)
            ar_out = dram_pool.tile([128, NM_O], f32, addr_space="Shared")
            nc.sync.dma_start(ar_in[:], p_sb[:])
            nc.gpsimd.collective_compute(
                "AllReduce",
                OP.add,
                replica_groups=[list(range(NCORES))],
                ins=[ar_in.opt()],
                outs=[ar_out.opt()],
            )
            hsum = pp.tile([128, NM_O], f32)
            nc.sync.dma_start(hsum[:], ar_out[:])
            hid = pp.tile([128, HK], f32)
            nc.vector.tensor_add(hid[:], x_sb[:], hsum[:])

            # ---------- post-LN + gated MLP ---------------------------
            h2 = pp.tile([128, HK], wdt)
            rmsnorm(hid, lnw2, h2, 1)

            ps_gu = ps_mm.tile([128, NM_GU], f32, tag="mm", name="ps_gu")
            for m in range(NM_GU):
                for k in range(HK):
                    nc.tensor.matmul(
                        ps_gu[:, m:m + 1],
                        wgu_t[m][:, k * 128:(k + 1) * 128],
                        h2[:, k:k + 1],
                        start=(k == 0),
                        stop=(k == HK - 1),
                    )
            sg = pp.tile([128, NK_D], f32)
            nc.scalar.activation(
                sg[:], ps_gu[:, 0:NK_D], AF.Silu, bias=zero_col[:, 0:1]
            )
            act = pp.tile([128, NK_D], wdt)
            nc.vector.tensor_mul(act[:], sg[:], ps_gu[:, NK_D:2 * NK_D])

            ps_m = ps_mm.tile([128, NM_O], f32, tag="mm", name="ps_m")
            for ch in range(16):
                for mm in range(2):
                    m = ch * 2 + mm
                    for k in range(NK_D):
                        nc.tensor.matmul(
                            ps_m[:, m:m + 1],
                            wd_t[ch][:, (mm * NK_D + k) * 128:(mm * NK_D + k + 1) * 128],
                            act[:, k:k + 1],
                            start=(k == 0),
                            stop=(k == NK_D - 1),
                        )

            # ---------- out = hid/8 + mlp_partial ---------------------
            out_sb = pp.tile([128, HK], f32)
            nc.vector.scalar_tensor_tensor(
                out_sb[:], hid[:], 1.0 / NCORES, ps_m[:], OP.mult, OP.add
            )
            nc.sync.dma_start(out_d[:], out_sb[:])

    nc.compile()
    return nc


def _get_nc():
    key = COMPUTE
    if key not in _cache:
        _cache[key] = _build()
    return _cache[key]


# ---------------------------------------------------------------- host prep
def _prepare(inputs):
    npw = _npwdt()
    f32 = np.float32

    x = np.asarray(inputs["hidden_conv"], f32).reshape(HID)
    cos = np.asarray(inputs["cos"], f32).reshape(D)
    sin = np.asarray(inputs["sin"], f32).reshape(D)
    cmask = np.asarray(inputs["causal_mask"], f32).reshape(S)
    pos = int(np.asarray(inputs["current_pos"]).reshape(1)[0])
    kv = np.asarray(inputs["kv_cache"], f32)            # (2, HKV, S, D)
    Wq = np.asarray(inputs["Wq"], f32)                  # (4096, 4096)
    Wk = np.asarray(inputs["Wk"], f32)                  # (1024, 4096)
    Wv = np.asarray(inputs["Wv"], f32)                  # (1024, 4096)
    Wo = np.asarray(inputs["Wo"], f32)                  # (4096, 4096)
    Wgu = np.asarray(inputs["Wgu"], f32)                # (24576, 4096)
    Wd = np.asarray(inputs["Wd"], f32)                  # (4096, 12288)
    in_ln = np.asarray(inputs["in_ln_w"], f32)
    post_ln = np.asarray(inputs["post_ln_w"], f32)
    qn = np.asarray(inputs["q_norm_w"], f32)
    kn = np.asarray(inputs["k_norm_w"], f32)

    def pf(v):  # (4096,) -> [128 part, 32]
        return np.ascontiguousarray(v.reshape(HK, 128).T)

    x_pf = pf(x)
    lnw1 = pf(in_ln)
    lnw2 = pf(post_ln)
    qknw = np.ascontiguousarray(
        np.concatenate([np.broadcast_to(qn, (4, D)), kn[None, :]], axis=0)
    )
    cos5 = np.ascontiguousarray(np.broadcast_to(cos, (5, D)))
    sin5 = np.ascontiguousarray(np.broadcast_to(sin, (5, D)))
    cmpos = np.full((1, 1), cmask[pos], f32)
    ident = np.eye(128, dtype=f32)

    # qkv slab: per core rows = [Wq 512 | Wk 128 | Wv 128]; slab[c][p, m, k, i]
    # = rows_c[128m + i, 128k + p]
    Wq6 = Wq.reshape(NCORES, 4, 128, HK, 128)     # [c, mt, i, k, p]
    Wk6 = Wk.reshape(NCORES, 1, 128, HK, 128)
    Wv6 = Wv.reshape(NCORES, 1, 128, HK, 128)
    qkv = np.concatenate([Wq6, Wk6, Wv6], axis=1)  # [c, m, i, k, p]
    wqkv = np.ascontiguousarray(qkv.transpose(0, 4, 1, 3, 2)).astype(npw)
    wqkv = wqkv.reshape(NCORES, 128, NM_QKV * HK * 128)

    # wo slab: slab[c][p, m, j, i] = Wo[128m + i, 512c + 128j + p]
    Wo5 = Wo.reshape(NM_O, 128, NCORES, NKO, 128)  # [m, i, c, j, p]
    wo = np.ascontiguousarray(Wo5.transpose(2, 4, 0, 3, 1)).astype(npw)
    wo = wo.reshape(NCORES, 128, NM_O * NKO * 128)

    # wgu slab: rows_c = [gate rows 1536c.. | up rows INT+1536c..]
    Wgu6 = Wgu.reshape(2, NCORES, NM_GU // 2, 128, HK, 128)  # [g, c, mt, i, k, p]
    wgu = np.ascontiguousarray(Wgu6.transpose(1, 5, 0, 2, 4, 3)).astype(npw)
    wgu = wgu.reshape(NCORES, 128, NM_GU * HK * 128)

    # wd slab: slab[c][p, m, kt, i] = Wd[128m + i, 1536c + 128kt + p]
    Wd5 = Wd.reshape(NM_O, 128, NCORES, NK_D, 128)  # [m, i, c, kt, p]
    wd = np.ascontiguousarray(Wd5.transpose(2, 4, 0, 3, 1)).astype(npw)
    wd = wd.reshape(NCORES, 128, NM_O * NK_D * 128)

    # k^T with column pos zeroed: kt[c] = kv[0, c].T, kt[:, pos] = 0
    kt = np.ascontiguousarray(kv[0].transpose(0, 2, 1))  # (8, 128, 8192)
    kt[:, :, pos] = 0.0
    kt = kt.astype(npw)

    # augmented v slab: [v*emask | emask | onehot*emask], v row pos zeroed
    emask = np.exp(cmask).astype(f32)                    # (S,)
    v = kv[1].copy()                                     # (8, 8192, 128)
    v[:, pos, :] = 0.0
    v = v * emask[None, :, None]
    v4 = v.reshape(NCORES, NS, 128, 128).transpose(0, 2, 1, 3)  # [c, p, j, d]
    onehot = np.zeros(S, f32)
    onehot[pos] = emask[pos]
    ecol = emask.reshape(NS, 128).T                      # [p, j]
    ocol = onehot.reshape(NS, 128).T
    vaug = np.empty((NCORES, 128, NS, 130), f32)
    vaug[..., 0:128] = v4
    vaug[..., 128] = ecol[None]
    vaug[..., 129] = ocol[None]
    vaug = vaug.astype(npw).reshape(NCORES, 128, NS * 130)

    in_maps = []
    for c in range(NCORES):
        in_maps.append({
            "x": x_pf, "lnw1": lnw1, "lnw2": lnw2, "qknw": qknw,
            "cos5": cos5, "sin5": sin5, "cmpos": cmpos, "ident": ident,
            "wqkv": wqkv[c], "wo": wo[c], "wgu": wgu[c], "wd": wd[c],
            "kt": kt[c], "vaug": vaug[c],
        })
    return in_maps


def _execute(inputs, trace=False, **kw):
    nc = _get_nc()
    in_maps = _prepare(inputs)
    res = run_bass_kernel_spmd(
        nc, in_maps, core_ids=list(range(NCORES)), trace=trace, **kw
    )
    acc = np.zeros((128, HK), np.float64)
    for c in range(NCORES):
        acc += np.asarray(res.results[c]["out"], np.float64)
    out = acc.T.reshape(1, HID, 1, 1).astype(np.float32)
    return out, res


def kernel(**inputs):
    out, _ = _execute(inputs)
    return out


# revision 17
# speedup vs baseline: 7.2722x; 7.2722x over previous
"""Trainium2 Bass kernel: single-token GQA decode layer + gated MLP,
Megatron-style tensor-parallel across 8 NeuronCores.

Self-contained: hardcodes all shapes. kernel(**inputs) takes the full
(unsharded) inputs and returns the full (1, HID, 1, 1) output.

Per-core shard (core c of 8):
  - attention: q heads [4c, 4c+4), kv head c, Wo columns [512c, 512c+512)
  - MLP: intermediate slice [1536c, 1536c+1536) of gate/up/down
  - one on-device AllReduce (f32, 16KB) after o_proj; the down_proj
    partials are summed on the host (each core returns hidden/8 + mlp_i).

All weights are pre-transposed + tiled on the host into [128, ...] slabs
with the contraction index on the partition dim, so every DMA descriptor
is a large contiguous run and every matmul is lhsT=[K=128, M=128] (or
the activation vector as rhs, N=1).

KV-cache scatter at current_pos is handled without any device-side
dynamic indexing: the host zeroes column pos of k^T and row pos of v,
bakes exp(causal_mask) and a one-hot(pos) column into an augmented
V slab [v*emask | emask | onehot*emask], and the device adds the new
token's contribution exp(q.k_new) (x) [v_new | 1 | 0] into the same
PSUM accumulator, then corrects the softmax denominator with the
one-hot column.
"""

import numpy as np
import ml_dtypes

import concourse.bass as bass
import concourse.bacc as bacc
import concourse.mybir as mybir
import concourse.tile as tile
from concourse.bass_utils import run_bass_kernel_spmd

# ---- problem dims (hardcoded) ----
H = 32
HKV = 8
G = H // HKV          # 4 q heads per kv head
D = 128
HID = 4096            # = 32 * 128
INT = 12288           # per-core intermediate = 1536 = 12 * 128
S = 8192              # = 64 * 128
EPS = 1e-6
SCALE = 1.0 / float(np.sqrt(D))
NCORES = 8

HK = HID // 128       # 32 k-tiles over hidden
QR = G * D            # 512 q rows per core
NM_QKV = (QR + 2 * D) // 128   # 6 m-tiles (4 q + 1 k + 1 v)
NM_O = HID // 128     # 32
NKO = QR // 128       # 4 k-tiles for o_proj
IPC = INT // NCORES   # 1536 per-core intermediate
NM_GU = 2 * IPC // 128  # 24 m-tiles (12 gate + 12 up)
NK_D = IPC // 128     # 12 k-tiles for down proj
NS = S // 128         # 64 s-tiles

# compute dtype for weights / big matmuls ("bf16" or "f32")
COMPUTE = "bf16"

_cache = {}


def _wdt():
    return mybir.dt.bfloat16 if COMPUTE == "bf16" else mybir.dt.float32


def _npwdt():
    return ml_dtypes.bfloat16 if COMPUTE == "bf16" else np.float32


# ---------------------------------------------------------------- builder
def _build(iters=1):
    f32 = mybir.dt.float32
    wdt = _wdt()
    bf = COMPUTE == "bf16"

    nc = bacc.Bacc(
        "TRN2",
        target_bir_lowering=False,
        debug=False,
        enable_asserts=False,
        num_devices=NCORES,
    )

    def inp(name, shape, dtype=f32):
        return nc.dram_tensor(name, list(shape), dtype, kind="ExternalInput").ap()

    x_d = inp("x", (128, HK))
    qkn_d = inp("qknw", (5, 128))
    cos_d = inp("cos5", (5, 128))
    sin_d = inp("sin5s", (5, 128))
    cmpos_d = inp("cmpos", (1, 1))
    ident_d = inp("ident", (128, 128))
    wqkv_d = inp("wqkv", (128, NM_QKV * HK * 128), wdt)
    wo_d = inp("wo", (128, NM_O * NKO * 128), wdt)
    wgu_d = inp("wgu", (128, NM_GU * HK * 128), wdt)
    wd_d = inp("wd", (128, NM_O * NK_D * 128), wdt)
    kt_d = inp("kt", (128, S), wdt)
    vaug_d = inp("vaug", (128, NS * 130), wdt)
    out_d = nc.dram_tensor("out", [128, HK], f32, kind="ExternalOutput").ap()

    AF = mybir.ActivationFunctionType
    OP = mybir.AluOpType

    with tile.TileContext(nc) as tc:
      for _it in range(iters):
        with (
            tc.tile_pool(name="persist", bufs=1) as pp,
            tc.tile_pool(name="stream", bufs=11) as sp,
            tc.tile_pool(name="e_s", bufs=4) as e_pool,
            tc.tile_pool(name="ps_small", bufs=2, space="PSUM") as ps_small,
            tc.tile_pool(name="ps_mm", bufs=2, space="PSUM") as ps_mm,
            tc.tile_pool(name="ps_sc", bufs=2, space="PSUM") as ps_sc_pool,
            tc.tile_pool(name="ps_acc", bufs=1, space="PSUM") as ps_acc_pool,
            tc.tile_pool(name="dram", bufs=1, space="DRAM") as dram_pool,
        ):
            # ---- small constant loads: ACT (scalar) HWDGE ring, so they
            # never queue behind the bulk weight stream on the sync ring
            x_sb = pp.tile([128, HK], f32)
            qknw = pp.tile([5, 128], f32)
            cos5 = pp.tile([5, 128], f32)
            sin5s = pp.tile([5, 128], f32)
            cmpos = pp.tile([1, 1], f32)
            ident = pp.tile([128, 128], f32)
            nc.scalar.dma_start(x_sb[:], x_d[:])
            nc.scalar.dma_start(qknw[:], qkn_d[:])
            nc.scalar.dma_start(cos5[:], cos_d[:])
            nc.scalar.dma_start(sin5s[:], sin_d[:])
            nc.scalar.dma_start(cmpos[:], cmpos_d[:])
            nc.scalar.dma_start(ident[:], ident_d[:])

            # ---- bulk streams: ONE shared pool (tag "w") so MLP chunks
            # time-multiplex the SBUF that attention chunks vacate.
            # All on the sync (SP) HWDGE ring, in consumption order:
            # qkv -> kT -> v_aug -> wo -> wgu -> wd. ~2MB chunks.
            QKV_CH = 2            # m-tiles per chunk (6 = 3 chunks)
            wqkv_t = []
            for ch in range(NM_QKV // QKV_CH):
                wq = sp.tile([128, QKV_CH * HK * 128], wdt, tag="w",
                             name=f"wqkv{ch}")
                nc.sync.dma_start(
                    wq[:],
                    wqkv_d[:, ch * QKV_CH * HK * 128:(ch + 1) * QKV_CH * HK * 128],
                )
                wqkv_t.append(wq)
            kt_sb = sp.tile([128, S], wdt, tag="w", name="kt_sb")
            nc.sync.dma_start(kt_sb[:], kt_d[:])
            va_sb = sp.tile([128, NS * 130], wdt, tag="w", name="va_sb")
            nc.sync.dma_start(va_sb[:], vaug_d[:])
            WO_CH = 16            # m-tiles per chunk (32 = 2 chunks)
            wo_t = []
            for ch in range(NM_O // WO_CH):
                wo = sp.tile([128, WO_CH * NKO * 128], wdt, tag="w",
                             name=f"wo{ch}")
                nc.sync.dma_start(
                    wo[:],
                    wo_d[:, ch * WO_CH * NKO * 128:(ch + 1) * WO_CH * NKO * 128],
                )
                wo_t.append(wo)
            GU_CH = 2             # m-tiles per chunk (24 = 12 chunks)
            wgu_t = []
            for ch in range(NM_GU // GU_CH):
                wg = sp.tile([128, GU_CH * HK * 128], wdt, tag="w",
                             name=f"wgu{ch}")
                nc.sync.dma_start(
                    wg[:],
                    wgu_d[:, ch * GU_CH * HK * 128:(ch + 1) * GU_CH * HK * 128],
                )
                wgu_t.append(wg)
            wd_sizes = [5, 5, 5, 5, 5, 5, 2]   # last chunk small -> short tail
            wd_t = []
            wd_off = 0
            for i, sz in enumerate(wd_sizes):
                wdl = sp.tile([128, sz * NK_D * 128], wdt, tag="w",
                              name=f"wd{i}")
                nc.sync.dma_start(
                    wdl[:],
                    wd_d[:, wd_off * NK_D * 128:(wd_off + sz) * NK_D * 128],
                )
                wd_t.append((wdl, wd_off, sz))
                wd_off += sz

            ones_col = pp.tile([128, 1], f32)
            ones_row = pp.tile([1, 128], f32)
            eps_col = pp.tile([128, 1], f32)
            zero_col = pp.tile([128, 1], f32)
            nc.vector.memset(ones_col[:], 1.0)
            nc.vector.memset(ones_row[:], 1.0)
            nc.vector.memset(eps_col[:], EPS)
            nc.vector.memset(zero_col[:], 0.0)

            # in_ln_w / post_ln_w are folded into the weight slabs on the
            # host, and rmsnorm is scale-invariant through the q/k norms,
            # so the projections consume the RAW x / hid and only the
            # 1/rms scalar is needed (for the v row and the MLP gate/up).
            # That keeps the norm chains off the critical path.
            def rms_inv(src, nbuf):
                scr = pp.tile([128, HK], f32, tag="rms_scr", name=f"scr{nbuf}")
                rs = pp.tile([128, 1], f32, tag="rms_rs", name=f"rs{nbuf}")
                nc.scalar.activation(
                    scr[:], src[:], AF.Square, bias=zero_col[:, 0:1],
                    accum_out=rs[:],
                )
                ps_tot = ps_small.tile([1, 1], f32, tag="small", name=f"pt{nbuf}")
                nc.tensor.matmul(ps_tot[:], rs[:], ones_col[:], start=True, stop=True)
                inv = pp.tile([1, 1], f32, tag="rms_inv", name=f"inv{nbuf}")
                nc.scalar.activation(
                    inv[:], ps_tot[:], AF.Abs_reciprocal_sqrt,
                    bias=eps_col[0:1, 0:1], scale=1.0 / HID,
                )
                return inv

            xb = pp.tile([128, HK], wdt)
            nc.vector.tensor_copy(xb[:], x_sb[:])
            inv1 = rms_inv(x_sb, 0)   # off critical path (only scales v_new)

            # ---- qkv projection (weights have in_ln_w folded in)
            ps_qkv = ps_mm.tile([128, NM_QKV], f32, tag="mm", name="ps_qkv")
            for ch in range(NM_QKV // QKV_CH):
                for mm in range(QKV_CH):
                    m = ch * QKV_CH + mm
                    for k in range(HK):
                        nc.tensor.matmul(
                            ps_qkv[:, m:m + 1],
                            wqkv_t[ch][:, (mm * HK + k) * 128:(mm * HK + k + 1) * 128],
                            xb[:, k:k + 1],
                            start=(k == 0),
                            stop=(k == HK - 1),
                        )

            # ---- transpose to [head, d], norm + rope
            qkvT = pp.tile([128, NM_QKV], f32)
            nc.vector.tensor_copy(qkvT[:], ps_qkv[:])
            ps_t1 = ps_small.tile([NM_QKV, 128], f32, tag="small", name="ps_t1")
            nc.tensor.transpose(ps_t1[:], qkvT[:], ident[:])
            qkv_hd = pp.tile([NM_QKV, 128], f32)
            nc.vector.tensor_copy(qkv_hd[:], ps_t1[:])

            # rmsnorm rows 0..4 (4 q heads + k) over free dim; the missing
            # 1/rms(x) factor cancels here (scale-invariance, eps-negligible)
            scr5 = pp.tile([5, 128], f32)
            rs5 = pp.tile([5, 1], f32)
            nc.scalar.activation(
                scr5[:], qkv_hd[0:5, :], AF.Square, bias=zero_col[0:5, 0:1],
                accum_out=rs5[:],
            )
            inv5 = pp.tile([5, 1], f32)
            nc.scalar.activation(
                inv5[:], rs5[:], AF.Abs_reciprocal_sqrt,
                bias=eps_col[0:5, 0:1], scale=1.0 / D,
            )
            # qkn2 = [qkn | qkn] so the rope rotation is a plain slice
            # qkn2[:, 64:192], with signs folded into sin5s on the host
            qkn2 = pp.tile([5, 256], f32)
            nc.vector.scalar_tensor_tensor(
                qkn2[:, 0:128], qkv_hd[0:5, :], inv5[:, 0:1], qknw[:],
                OP.mult, OP.mult,
            )
            nc.vector.tensor_copy(qkn2[:, 128:256], qkn2[:, 0:128])
            t1 = pp.tile([5, 128], f32)
            nc.vector.tensor_mul(t1[:], qkn2[:, 0:128], cos5[:])
            qk_r = pp.tile([5, 128], f32)
            nc.vector.tensor_mul(qk_r[:], qkn2[:, 64:192], sin5s[:])
            nc.vector.tensor_add(qk_r[:], qk_r[:], t1[:])

            # transpose back -> qkT [d=128, 5] (cols 0-3 q heads, col 4 k_new)
            ps_t2 = ps_small.tile([128, 5], f32, tag="small", name="ps_t2")
            nc.tensor.transpose(ps_t2[:], qk_r[:], ident[0:5, 0:5])
            qkT = pp.tile([128, 5], wdt)
            nc.vector.tensor_copy(qkT[:], ps_t2[:])

            # vnew_aug [1, 130] = [inv1 * v_raw | 1 | 0]; extract the v row
            # via a transpose of qkvT column 5 (partition slices must be
            # 32-aligned, so qkv_hd[5:6, :] cannot be read directly)
            ps_vt = ps_small.tile([1, 128], f32, tag="small", name="ps_vt")
            nc.tensor.transpose(ps_vt[:], qkvT[:, 5:6], ident[:])
            vnew = pp.tile([1, 130], wdt)
            nc.vector.memset(vnew[:], 0.0)
            nc.vector.tensor_scalar_mul(vnew[0:1, 0:128], ps_vt[:], inv1[0:1, 0:1])
            nc.vector.memset(vnew[0:1, 128:129], 1.0)

            # e_new^T [1, 4] = exp(SCALE * q.k_new + cm[pos])
            ps_en = ps_small.tile([1, 4], f32, tag="small", name="ps_en")
            nc.tensor.matmul(
                ps_en[:], qkT[:, 4:5], qkT[:, 0:4], start=True, stop=True
            )
            enT = pp.tile([1, 4], wdt)
            nc.scalar.activation(
                enT[:], ps_en[:], AF.Exp, bias=cmpos[0:1, 0:1], scale=SCALE
            )

            # ---- scores + exp (16 s-tiles per PSUM batch)
            e_t = []
            for b in range(4):
                ps_sc = ps_sc_pool.tile([128, 64], f32, tag="sc", name=f"sc{b}")
                for t in range(16):
                    j = b * 16 + t
                    nc.tensor.matmul(
                        ps_sc[:, 4 * t:4 * t + 4],
                        kt_sb[:, j * 128:(j + 1) * 128],
                        qkT[:, 0:4],
                        start=True,
                        stop=True,
                    )
                e_sb = e_pool.tile([128, 64], wdt, tag="e_c", name=f"e{b}")
                nc.scalar.activation(
                    e_sb[:], ps_sc[:], AF.Exp, bias=zero_col[:, 0:1], scale=SCALE
                )
                e_t.append(e_sb)

            # ---- attn @ v_aug
            ps_acc = ps_acc_pool.tile([4, 130], f32)
            first = True
            for b in range(4):
                for t in range(16):
                    j = b * 16 + t
                    nc.tensor.matmul(
                        ps_acc[:],
                        e_t[b][:, 4 * t:4 * t + 4],
                        va_sb[:, j * 130:(j + 1) * 130],
                        start=first,
                        stop=False,
                    )
                    first = False
            nc.tensor.matmul(ps_acc[:], enT[:], vnew[:], start=False, stop=True)

            acc_sb = pp.tile([4, 130], f32)
            nc.vector.tensor_copy(acc_sb[:], ps_acc[:])
            den = pp.tile([4, 1], f32)
            nc.vector.tensor_sub(den[:], acc_sb[:, 128:129], acc_sb[:, 129:130])
            rec = pp.tile([4, 1], f32)
            nc.vector.reciprocal(rec[:], den[:])
            attn = pp.tile([4, 128], f32)
            nc.vector.tensor_scalar_mul(attn[:], acc_sb[:, 0:128], rec[:, 0:1])
            ps_t3 = ps_small.tile([128, 4], f32, tag="small", name="ps_t3")
            nc.tensor.transpose(ps_t3[:], attn[:], ident[0:4, 0:4])
            attnT = pp.tile([128, 4], wdt)
            nc.vector.tensor_copy(attnT[:], ps_t3[:])

            # ---- o_proj partial
            ps_p = ps_mm.tile([128, NM_O], f32, tag="mm", name="ps_p")
            for ch in range(NM_O // WO_CH):
                for mm in range(WO_CH):
                    m = ch * WO_CH + mm
                    for j in range(NKO):
                        nc.tensor.matmul(
                            ps_p[:, m:m + 1],
                            wo_t[ch][:, (mm * NKO + j) * 128:(mm * NKO + j + 1) * 128],
                            attnT[:, j:j + 1],
                            start=(j == 0),
                            stop=(j == NKO - 1),
                        )
            p_sb = pp.tile([128, NM_O], f32)
            nc.vector.tensor_copy(p_sb[:], ps_p[:])

            # ---- AllReduce o_proj partials (ACT ring for the DMAs)
            ar_in = dram_pool.tile([128, NM_O], f32)
            ar_out = dram_pool.tile([128, NM_O], f32, addr_space="Shared")
            nc.scalar.dma_start(ar_in[:], p_sb[:])
            nc.gpsimd.collective_compute(
                "AllReduce",
                OP.add,
                replica_groups=[list(range(NCORES))],
                ins=[ar_in.opt()],
                outs=[ar_out.opt()],
            )
            hsum = pp.tile([128, NM_O], f32)
            nc.scalar.dma_start(hsum[:], ar_out[:])
            hid = pp.tile([128, HK], f32)
            nc.vector.tensor_add(hid[:], x_sb[:], hsum[:])
            hidb = pp.tile([128, HK], wdt)
            nc.vector.tensor_copy(hidb[:], hid[:])

            # 1/rms(hid): runs in parallel with the wgu matmuls; applied
            # via the Silu activation's scale operand and the act multiply
            inv2 = rms_inv(hid, 1)
            ps_b2 = ps_small.tile([128, 1], f32, tag="small", name="pb2")
            nc.tensor.matmul(ps_b2[:], ones_row[:], inv2[:], start=True, stop=True)
            invb2 = pp.tile([128, 1], f32)
            nc.vector.tensor_copy(invb2[:], ps_b2[:])

            # ---- gated MLP (weights have post_ln_w folded in)
            ps_gu = ps_mm.tile([128, NM_GU], f32, tag="mm", name="ps_gu")
            for ch in range(NM_GU // GU_CH):
                for mm in range(GU_CH):
                    m = ch * GU_CH + mm
                    for k in range(HK):
                        nc.tensor.matmul(
                            ps_gu[:, m:m + 1],
                            wgu_t[ch][:, (mm * HK + k) * 128:(mm * HK + k + 1) * 128],
                            hidb[:, k:k + 1],
                            start=(k == 0),
                            stop=(k == HK - 1),
                        )
            sg = pp.tile([128, NK_D], f32)
            nc.scalar.activation(
                sg[:], ps_gu[:, 0:NK_D], AF.Silu, bias=zero_col[:, 0:1],
                scale=invb2[:, 0:1],
            )
            act = pp.tile([128, NK_D], wdt)
            # act = (up * inv2) * silu(inv2 * gate)
            nc.vector.scalar_tensor_tensor(
                act[:], ps_gu[:, NK_D:2 * NK_D], invb2[:, 0:1], sg[:],
                OP.mult, OP.mult,
            )

            ps_m = ps_mm.tile([128, NM_O], f32, tag="mm", name="ps_m")
            for wdl, off, sz in wd_t:
                for mm in range(sz):
                    m = off + mm
                    for k in range(NK_D):
                        nc.tensor.matmul(
                            ps_m[:, m:m + 1],
                            wdl[:, (mm * NK_D + k) * 128:(mm * NK_D + k + 1) * 128],
                            act[:, k:k + 1],
                            start=(k == 0),
                            stop=(k == NK_D - 1),
                        )

            # ---- out = hid/8 + mlp_partial
            out_sb = pp.tile([128, HK], f32)
            nc.vector.scalar_tensor_tensor(
                out_sb[:], hid[:], 1.0 / NCORES, ps_m[:], OP.mult, OP.add
            )
            nc.scalar.dma_start(out_d[:], out_sb[:])

    nc.compile()
    return nc


def _get_nc():
    key = COMPUTE
    if key not in _cache:
        _cache[key] = _build()
    return _cache[key]


# ---------------------------------------------------------------- host prep
def _prepare(inputs):
    npw = _npwdt()
    f32 = np.float32

    x = np.asarray(inputs["hidden_conv"], f32).reshape(HID)
    cos = np.asarray(inputs["cos"], f32).reshape(D)
    sin = np.asarray(inputs["sin"], f32).reshape(D)
    cmask = np.asarray(inputs["causal_mask"], f32).reshape(S)
    pos = int(np.asarray(inputs["current_pos"]).reshape(1)[0])
    kv = np.asarray(inputs["kv_cache"], f32)            # (2, HKV, S, D)
    Wq = np.asarray(inputs["Wq"], f32)                  # (4096, 4096)
    Wk = np.asarray(inputs["Wk"], f32)                  # (1024, 4096)
    Wv = np.asarray(inputs["Wv"], f32)                  # (1024, 4096)
    Wo = np.asarray(inputs["Wo"], f32)                  # (4096, 4096)
    Wgu = np.asarray(inputs["Wgu"], f32)                # (24576, 4096)
    Wd = np.asarray(inputs["Wd"], f32)                  # (4096, 12288)
    in_ln = np.asarray(inputs["in_ln_w"], f32)
    post_ln = np.asarray(inputs["post_ln_w"], f32)
    qn = np.asarray(inputs["q_norm_w"], f32)
    kn = np.asarray(inputs["k_norm_w"], f32)

    def pf(v):  # (4096,) -> [128 part, 32]
        return np.ascontiguousarray(v.reshape(HK, 128).T)

    x_pf = pf(x)
    qknw = np.ascontiguousarray(
        np.concatenate([np.broadcast_to(qn, (4, D)), kn[None, :]], axis=0)
    )
    cos5 = np.ascontiguousarray(np.broadcast_to(cos, (5, D)))
    # rope rotation as a 64-shifted slice: signs folded into sin
    sin_s = np.concatenate([-sin[:64], sin[64:]])
    sin5s = np.ascontiguousarray(np.broadcast_to(sin_s, (5, D)))
    cmpos = np.full((1, 1), cmask[pos], f32)
    ident = np.eye(128, dtype=f32)

    # fold the (scale-invariant-modulo-eps) layernorm weights into the
    # projection weight columns so qkv/MLP consume raw x / hid
    Wq = Wq * in_ln[None, :]
    Wk = Wk * in_ln[None, :]
    Wv = Wv * in_ln[None, :]
    Wgu = Wgu * post_ln[None, :]

    # qkv slab: per core rows = [Wq 512 | Wk 128 | Wv 128]; slab[c][p, m, k, i]
    # = rows_c[128m + i, 128k + p]
    Wq6 = Wq.reshape(NCORES, 4, 128, HK, 128)     # [c, mt, i, k, p]
    Wk6 = Wk.reshape(NCORES, 1, 128, HK, 128)
    Wv6 = Wv.reshape(NCORES, 1, 128, HK, 128)
    qkv = np.concatenate([Wq6, Wk6, Wv6], axis=1)  # [c, m, i, k, p]
    wqkv = np.ascontiguousarray(qkv.transpose(0, 4, 1, 3, 2)).astype(npw)
    wqkv = wqkv.reshape(NCORES, 128, NM_QKV * HK * 128)

    # wo slab: slab[c][p, m, j, i] = Wo[128m + i, 512c + 128j + p]
    Wo5 = Wo.reshape(NM_O, 128, NCORES, NKO, 128)  # [m, i, c, j, p]
    wo = np.ascontiguousarray(Wo5.transpose(2, 4, 0, 3, 1)).astype(npw)
    wo = wo.reshape(NCORES, 128, NM_O * NKO * 128)

    # wgu slab: rows_c = [gate rows 1536c.. | up rows INT+1536c..]
    Wgu6 = Wgu.reshape(2, NCORES, NM_GU // 2, 128, HK, 128)  # [g, c, mt, i, k, p]
    wgu = np.ascontiguousarray(Wgu6.transpose(1, 5, 0, 2, 4, 3)).astype(npw)
    wgu = wgu.reshape(NCORES, 128, NM_GU * HK * 128)

    # wd slab: slab[c][p, m, kt, i] = Wd[128m + i, 1536c + 128kt + p]
    Wd5 = Wd.reshape(NM_O, 128, NCORES, NK_D, 128)  # [m, i, c, kt, p]
    wd = np.ascontiguousarray(Wd5.transpose(2, 4, 0, 3, 1)).astype(npw)
    wd = wd.reshape(NCORES, 128, NM_O * NK_D * 128)

    # k^T with column pos zeroed: kt[c] = kv[0, c].T, kt[:, pos] = 0
    kt = np.ascontiguousarray(kv[0].transpose(0, 2, 1))  # (8, 128, 8192)
    kt[:, :, pos] = 0.0
    kt = kt.astype(npw)

    # augmented v slab: [v*emask | emask | onehot*emask], v row pos zeroed
    emask = np.exp(cmask).astype(f32)                    # (S,)
    v = kv[1].copy()                                     # (8, 8192, 128)
    v[:, pos, :] = 0.0
    v = v * emask[None, :, None]
    v4 = v.reshape(NCORES, NS, 128, 128).transpose(0, 2, 1, 3)  # [c, p, j, d]
    onehot = np.zeros(S, f32)
    onehot[pos] = emask[pos]
    ecol = emask.reshape(NS, 128).T                      # [p, j]
    ocol = onehot.reshape(NS, 128).T
    vaug = np.empty((NCORES, 128, NS, 130), f32)
    vaug[..., 0:128] = v4
    vaug[..., 128] = ecol[None]
    vaug[..., 129] = ocol[None]
    vaug = vaug.astype(npw).reshape(NCORES, 128, NS * 130)

    in_maps = []
    for c in range(NCORES):
        in_maps.append({
            "x": x_pf, "qknw": qknw,
            "cos5": cos5, "sin5s": sin5s, "cmpos": cmpos, "ident": ident,
            "wqkv": wqkv[c], "wo": wo[c], "wgu": wgu[c], "wd": wd[c],
            "kt": kt[c], "vaug": vaug[c],
        })
    return in_maps


def _execute(inputs, trace=False, **kw):
    nc = _get_nc()
    in_maps = _prepare(inputs)
    res = run_bass_kernel_spmd(
        nc, in_maps, core_ids=list(range(NCORES)), trace=trace, **kw
    )
    acc = np.zeros((128, HK), np.float64)
    for c in range(NCORES):
        acc += np.asarray(res.results[c]["out"], np.float64)
    out = acc.T.reshape(1, HID, 1, 1).astype(np.float32)
    return out, res


def kernel(**inputs):
    out, _ = _execute(inputs)
    return out
